# revision 36
# baseline (speedup 1.0000x reference)
"""Trainium2 Bass kernel for nn_CBFLayer (batch CBF-QP safety filter).

Contract: kernel(u_nom, obs) takes FULL inputs (numpy), returns FULL output.
Internally: pure data-parallel shard of the batch across 8 NeuronCores.

Math (per sample, exact KKT of the QP  min |u-u_nom|^2 + LAM*s^2
s.t. a@u <= b+s, |u|^2 <= 1, s >= 0, with a = -2*g, g = p_rel):
  u = (u_nom + 2*t*g) * rho,  rho = min(1/||u_nom + 2*t*g||, 1)
with multiplier t per KKT case: t=0 (feasible), t2 (CBF active, ball
inactive; exact linear root), or the closed-form circle root z =
c*|w|/sqrt(1-c^2) pole-floored by (|C|/(2LAM))/S plus a deep-infeasible
branch t = 2*LAM*relu(-(b/2+sqrt(S))).  Seed-only accuracy ~7e-4 rel;
bf16 data path lands ~6e-3, inside the 2e-2 gate.

Implementation notes:
- inputs shipped bf16 in per-tile blocks [gx gy | gy -gx | ux uy | vx vy]
  so products fuse into wide DVE ops via broadcast/strided views;
- all transcendentals are Sqrt/Square/Relu/Abs/Copy activations (one
  table set: sqrt_and_others); divisions via reciprocal_approx_fast
  (f32-only custom DVE op - the only f32 islands in the pipeline);
- affine+mul chains fused with scalar_tensor_tensor (DVE-only opcode).
"""

import numpy as np
from ml_dtypes import bfloat16

B = 4194304
NCORES = 8
BC = B // NCORES            # 524288 samples per core
P = 128
NPER = BC // P              # 4096 samples per partition
KC = 1024                   # compute-tile samples per partition
NT = NPER // KC             # tiles per core

LAM = 10000.0
TOL = 1e-6

_CACHE = {}


def _build():
    import bass_rust as _bass_rust
    import concourse.bacc as bacc
    import concourse.mybir as mybir
    from concourse.tile import TileContext
    from concourse.hw_specs import get_activation_tables

    F32 = mybir.dt.float32
    BF16 = mybir.dt.bfloat16
    OP = mybir.AluOpType
    AF = mybir.ActivationFunctionType

    class _PinnedBacc(bacc.Bacc):
        """Activation-table chooser only sees sqrt_and_others (list order
        preserved so act_func_set_id indices stay aligned)."""

        def insert_act_table_loads(self):
            has_activation = any(
                isinstance(i, mybir.InstActivation)
                for b in self.main_func.blocks
                for i in b.instructions
            )
            if not has_activation:
                return
            tables = [
                (k, v if k == "sqrt_and_others" else set())
                for k, v in get_activation_tables(self.m.arch).items()
            ]
            _bass_rust.insert_act_table_loads(self, tables)

    nc = _PinnedBacc("TRN2", target_bir_lowering=False, debug=False)
    pk_in = nc.dram_tensor("pk", [P, NPER * 8], BF16, kind="ExternalInput").ap()
    out_d = nc.dram_tensor("out", [P, NPER * 2], BF16, kind="ExternalOutput").ap()

    def register_const(value):
        t = nc.alloc_sbuf_tensor(f"const-f32-{value}", [P, 1], F32)
        nc.gpsimd.memset(t.ap(), value)
        nc.const_aps.aps[(F32, value)] = t.ap()

    register_const(0.0)
    register_const(-1.0)
    register_const(1.0)
    register_const(-0.5 * TOL)
    nc.all_engine_barrier()

    with TileContext(nc) as tc:
        with (
            tc.tile_pool(name="io", bufs=2) as io,
            tc.tile_pool(name="wk", bufs=2) as wk,       # cross-stage values
            tc.tile_pool(name="ck", bufs=1) as ck,       # short-lived scratch
        ):
            def eng(e):
                return {"V": nc.vector, "G": nc.gpsimd}[e]

            def tt(e, out, a, b, op):
                eng(e).tensor_tensor(out, a, b, op)

            def ts(e, out, a, s1, op0, s2=None, op1=None):
                if op1 is None:
                    eng(e).tensor_scalar(out, a, s1, None, op0)
                else:
                    eng(e).tensor_scalar(out, a, s1, s2, op0, op1)

            def stt(out, in0, s, in1, op0, op1):
                nc.vector.scalar_tensor_tensor(out, in0, s, in1, op0, op1)

            def act(out, a, func, scale=1.0, bias=0.0):
                nc.scalar.activation(out, a, func, bias=bias, scale=scale)

            def rcp(out, in_):
                nc.vector.reciprocal_approx_fast(out=out, in_=in_)

            def bcast(ap, n):
                return ap.rearrange("p (o b) -> p o b", o=1).broadcast_to([P, 2, n])

            def T(name, n, dt):
                return ck.tile([P, n], dt, tag=name, name=name)

            def TW(name, n, dt):
                return wk.tile([P, n], dt, tag=name, name=name)

            def stage_a(i):
                # blocks [Gx Gy | Gy -Gx | ux uy | vx/2 vy/2], G = 2*p_rel
                st = {}
                pk_t = io.tile([P, 8 * KC], BF16, tag="pk_t")
                o_t = io.tile([P, 2 * KC], BF16, tag="o_t")
                nc.sync.dma_start(out=pk_t[:], in_=pk_in[:, i * 8 * KC:(i + 1) * 8 * KC])
                st["pk_t"], st["o_t"] = pk_t, o_t
                gsb = pk_t[:, 0:4 * KC]
                gb = pk_t[:, 0:2 * KC]
                ub = pk_t[:, 4 * KC:6 * KC]
                vb = pk_t[:, 6 * KC:8 * KC]
                st["gb"], st["ub"] = gb, ub
                # products: S'=|G|^2=4S, P'=G.u=2P, C'=GxU=2C, Vd=g.v, N=|u|^2
                gucr4 = T("gucr4", 4 * KC, BF16)
                tt("V", gucr4[:].rearrange("p (a b) -> p a b", a=2),
                   gsb.rearrange("p (a b) -> p a b", a=2), bcast(ub, 2 * KC), OP.mult)
                sq4 = T("sq4", 4 * KC, BF16)
                act(sq4[:].rearrange("p (a b) -> p a b", a=2),
                    pk_t[:].rearrange("p (a b) -> p a b", a=4)[:, 0::2, :], AF.Square)
                SN = TW("SN", 2 * KC, BF16)
                PC = TW("PC", 2 * KC, BF16)
                st["SN"], st["PC"] = SN, PC
                tt("V", SN[:, 0:KC], sq4[:, 0:KC], sq4[:, KC:2 * KC], OP.add)
                tt("V", SN[:, KC:2 * KC], sq4[:, 2 * KC:3 * KC], sq4[:, 3 * KC:4 * KC], OP.add)
                tt("V", PC[:, 0:KC], gucr4[:, 0:KC], gucr4[:, KC:2 * KC], OP.add)
                tt("V", PC[:, KC:2 * KC], gucr4[:, 2 * KC:3 * KC], gucr4[:, 3 * KC:4 * KC], OP.add)
                gvb = T("gvb", 2 * KC, BF16)
                tt("V", gvb[:], gb, vb, OP.mult)
                Vd = TW("Vd", KC, BF16)
                tt("G", Vd[:], gvb[:, 0:KC], gvb[:, KC:2 * KC], OP.add)
                st["Vd"] = Vd
                return st

            def solve(i, st):
                SN, PC, Vd = st["SN"], st["PC"], st["Vd"]
                gb, ub, o_t = st["gb"], st["ub"], st["o_t"]
                S_ = SN[:, 0:KC]
                N_ = SN[:, KC:2 * KC]
                P_ = PC[:, 0:KC]
                C_ = PC[:, KC:2 * KC]
                # ---- emission order tuned from gap profile: chain-critical
                # producers first per engine, off-chain masks late ----
                S2x = T("S2x", 2 * KC, F32)
                act(S2x[:, 0:KC], S_, AF.Copy)
                act(S2x[:, KC:2 * KC], S_, AF.Copy, bias=1e-4)
                S4m = T("S4m", KC, BF16); act(S4m[:], S_, AF.Copy, scale=0.25, bias=-1.0)
                b1 = T("b1", KC, BF16); tt("V", b1[:], S4m[:], Vd[:], OP.subtract)  # b/2
                rcp2 = T("rcp2", 2 * KC, F32); rcp(rcp2[:], S2x[:])
                rS = rcp2[:, 0:KC]
                rden = rcp2[:, KC:2 * KC]
                # S fillers while rcp runs, then post-rcp converts
                Nc = T("Nc", KC, BF16); act(Nc[:], N_, AF.Relu, bias=-1.0)
                sqNc = T("sqNc", KC, BF16); act(sqNc[:], Nc[:], AF.Sqrt, bias=1.0)
                b2t = T("b2t", KC, BF16); act(b2t[:], b1[:], AF.Copy, scale=2.0)    # b
                rSb = T("rSb", KC, BF16); act(rSb[:], rS, AF.Copy, scale=1.0 / LAM)
                rdnb = T("rdnb", KC, BF16); act(rdnb[:], rden, AF.Copy, scale=-1.0)
                isq = T("isq", KC, BF16); act(isq[:], rS, AF.Sqrt, scale=4.0)
                rSbn = T("rSbn", KC, BF16); act(rSbn[:], rS, AF.Copy, scale=-1.0)
                sqS = T("sqS", KC, BF16); act(sqS[:], S_, AF.Sqrt, scale=0.25)
                acr = T("acr", KC, BF16); act(acr[:], C_, AF.Abs)
                # G early: feas pieces + case-2 numerator
                rhs = T("rhs", KC, BF16); tt("G", rhs[:], b2t[:], sqNc[:], OP.mult)
                ff = T("ff", KC, BF16); tt("G", ff[:], P_, rhs[:], OP.add)
                num = T("num", KC, BF16); tt("G", num[:], P_, b2t[:], OP.add)
                # case-2 chain
                t2 = T("t2", KC, BF16); tt("V", t2[:], num[:], rdnb[:], OP.mult)
                w_ = T("w_", KC, BF16); tt("V", w_[:], t2[:], S_, OP.mult)
                P2t = T("P2t", KC, BF16); act(P2t[:], P_, AF.Copy, scale=2.0)
                w2 = T("w2", KC, BF16); tt("G", w2[:], P2t[:], w_[:], OP.add)
                x2 = T("x2", KC, BF16); tt("V", x2[:], t2[:], w2[:], OP.mult)
                n2 = T("n2", KC, BF16); tt("G", n2[:], x2[:], N_, OP.add)
                # seed chain
                beta = T("beta", KC, BF16); tt("V", beta[:], b1[:], isq[:], OP.mult)
                bsq = T("bsq", KC, BF16); act(bsq[:], beta[:], AF.Square)
                w2m = T("w2m", KC, BF16); act(w2m[:], bsq[:], AF.Copy, scale=-1.0, bias=1.0)
                ws2 = T("ws2", KC, BF16); tt("V", ws2[:], acr[:], rSb[:], OP.mult)
                w2c = T("w2c", KC, F32); stt(w2c[:], w2m[:], 1e-12, ws2[:], OP.max, OP.max)
                iw = T("iw", KC, F32); rcp(iw[:], w2c[:])
                rw = T("rw", KC, BF16); act(rw[:], iw[:], AF.Sqrt)
                ta1 = T("ta1", KC, BF16); tt("G", ta1[:], b1[:], sqS[:], OP.add)
                # off-chain masks fill V while Scalar computes rw
                tq = T("w_", KC, BF16); act(tq[:], t2[:], AF.Copy, scale=-1e12)
                q1 = T("num", KC, BF16); tt("V", q1[:], tq[:], n2[:], OP.max)
                ok2 = T("ok2", KC, BF16); ts("V", ok2[:], q1[:], 1.0 + TOL, OP.is_le)
                nf1 = T("nf1", KC, BF16); ts("V", nf1[:], ff[:], -0.5 * TOL, OP.is_lt)
                km = T("km", KC, BF16); tt("V", km[:], acr[:], rw[:], OP.mult)
                km2 = T("km2", KC, BF16); tt("V", km2[:], km[:], beta[:], OP.mult)
                sm = T("sm", KC, BF16); tt("V", sm[:], P_, km2[:], OP.add)
                # select
                tmain = T("tmain", KC, BF16); tt("V", tmain[:], sm[:], rSbn[:], OP.mult)
                talt = T("talt", KC, BF16); act(talt[:], ta1[:], AF.Relu, scale=-2.0 * LAM)
                t = T("t", KC, BF16); tt("V", t[:], tmain[:], talt[:], OP.max)
                nc.vector.copy_predicated(t[:], ok2[:].bitcast(mybir.dt.uint16), t2[:])
                tt("V", t[:], t[:], nf1[:], OP.mult)
                st["t"] = t

            def solve2(i, st):
                gb, ub, o_t = st["gb"], st["ub"], st["o_t"]
                t, nf1 = st["t"], None
                axy = T("axy", 2 * KC, BF16)
                tt("V", axy[:].rearrange("p (o b) -> p o b", o=2),
                   bcast(t[:], KC), gb.rearrange("p (o b) -> p o b", o=2), OP.mult)
                sxy = TW("sxy", 2 * KC, BF16); tt("V", sxy[:], ub, axy[:], OP.add)
                sq2 = T("sq2", 2 * KC, BF16); tt("V", sq2[:], sxy[:], sxy[:], OP.mult)
                nnf = T("nnf", KC, F32)
                stt(nnf[:], sq2[:, 0:KC], 1e-30, sq2[:, KC:2 * KC], OP.add, OP.add)
                inf = T("inf", KC, F32); rcp(inf[:], nnf[:])
                rho0 = T("rho0", KC, BF16); act(rho0[:], inf[:], AF.Sqrt)
                rho = TW("rho", KC, BF16); ts("V", rho[:], rho0[:], 1.0, OP.min)
                tt("V", o_t[:].rearrange("p (o b) -> p o b", o=2),
                   sxy[:].rearrange("p (o b) -> p o b", o=2), bcast(rho[:], KC), OP.mult)
                nc.gpsimd.dma_start(out=out_d[:, i * 2 * KC:(i + 1) * 2 * KC], in_=o_t[:])

            sts = {0: stage_a(0)}
            for i in range(NT):
                solve(i, sts[i])
                if i + 1 < NT:
                    sts[i + 1] = stage_a(i + 1)
                solve2(i, sts.pop(i))
    nc.compile()
    return nc


def _get_nc():
    if "nc" not in _CACHE:
        _CACHE["nc"] = _build()
    return _CACHE["nc"]


def _run(u_nom: np.ndarray, obs: np.ndarray, trace: bool = False):
    from concourse.bass_utils import run_bass_kernel_spmd

    u_nom = np.asarray(u_nom, dtype=np.float32)
    obs = np.asarray(obs, dtype=np.float32)

    nc = _get_nc()
    in_maps = []
    for c in range(NCORES):
        s = slice(c * BC, (c + 1) * BC)
        uc = u_nom[s].reshape(P, NT, KC, 2).astype(bfloat16)
        oc = obs[s].reshape(P, NT, KC, 6).astype(bfloat16)
        gx = (2.0 * oc[:, :, :, 2].astype(np.float32)).astype(bfloat16)
        gy = (2.0 * oc[:, :, :, 3].astype(np.float32)).astype(bfloat16)
        # blocks: [Gx Gy | Gy -Gx | ux uy | vx/2 vy/2], G = 2*p_rel
        pk = np.stack(
            [gx, gy, gy, -gx,
             uc[:, :, :, 0], uc[:, :, :, 1],
             (0.5 * oc[:, :, :, 4].astype(np.float32)).astype(bfloat16),
             (0.5 * oc[:, :, :, 5].astype(np.float32)).astype(bfloat16)],
            axis=2).reshape(P, NPER * 8)
        in_maps.append({"pk": np.ascontiguousarray(pk)})
    res = run_bass_kernel_spmd(nc, in_maps, core_ids=list(range(NCORES)),
                               trace=trace)
    out = np.empty((B, 2), dtype=np.float32)
    for c in range(NCORES):
        r = np.asarray(res.results[c]["out"]).view(bfloat16).astype(np.float32)
        r = r.reshape(P, NT, 2, KC)
        out[c * BC:(c + 1) * BC] = np.transpose(r, (0, 1, 3, 2)).reshape(BC, 2)
    return out, res


def kernel(u_nom: np.ndarray, obs: np.ndarray) -> np.ndarray:
    return _run(u_nom, obs)[0]


if __name__ == "__main__":
    rng = np.random.default_rng(0)
    u = rng.standard_normal((B, 2), dtype=np.float32)
    o = rng.standard_normal((B, 6), dtype=np.float32)
    r = kernel(u, o)
    print(r.shape, r.dtype, r[:4])
